# revision 68
# baseline (speedup 1.0000x reference)
"""GQA dense-transformer kernel for 8 Trainium2 NeuronCores.

Problem (hardcoded): B=2, S=2048, D=2048, kv_heads=16, groups G=4, HPG=4,
HD=128.  reference:
    qkv = x @ Wqkv + bqkv ; q,k,v = split(qkv)
    q = einsum('bsghd,gde->bsghe', q, Wq) + bq   (per-group shared proj)
    v = einsum('bsghd,gde->bsghe', v, Wv) + bv
    scores = einsum('bqghd,bkghd->bghqk', q, k) / sqrt(HD)
    attn = softmax(scores) * attn_mask           (mask == ones at grading)
    out = einsum('bghqk,bkghd->bqghd', attn, v)  -> [B,S,D]

Sharding: core c = b*4 + g handles (batch b, group g): it computes the
512 output columns [g*512,(g+1)*512) of out[b].

Per-core device program (bf16 matmuls, fp32 PSUM):
  BOTH per-group projections are folded into Wqkv on the HOST:
    W_q2'[:,hs] = Wqkv_q[:,hs] @ (Wq[g]*SCALE)   (query path + attn scale)
    W_v2'[:,hs] = Wqkv_v[:,hs] @ Wv[g]           (value path)
  so the device computes q2^T, k^T and v2 directly in one projection
  pass.  Inputs are RELAID OUT partition-major on the host so every DMA
  descriptor covers a 1-16KB contiguous run.
  phase 1: chunk 0 (s in [0,512)) runs K-OUTER in groups of 4 k-blocks
           with 8 live PSUM banks (m=0..7 = q2',k); v2 and chunks 1..3
           run m-outer.  v2 is computed in NATURAL (key-partition)
           layout directly: per 128-key block, lhsT = the xT key slice
           (stationary), rhs = W_v2' k-slab, PSUM-accumulated over the
           16 k-blocks.  q2/k evacuate on DVE (tensor_scalar bias add),
           v2 on DVE, keeping ACT free for early attention.
  early attention: scores+exp for heads 0/1, k-blocks j<12 (chunks
           0..2), query columns [0,1536) are INTERLEAVED into chunk 3's
           projection window, so the ACT engine (the phase-2
           bottleneck) starts ~60us early and phase 2 shrinks.
  phase 2: a single SLOT SCHEDULER walks (head, j) score slots -- for
           early (h,j) only the last query chunk remains -- and weaves
           the global PV queue (4 col-chunk matmuls per (h,j), j-MAJOR
           PSUM accumulation, 2-slot lag behind the exps) plus the
           denominator accumulation into the stream, so the PE and DVE
           track ACT with no per-head barrier.
           Denominators: dacc[h] += P_j on the DVE (bf16, 2x rate),
           column-summed by 4 CONCURRENT M=1 ones-matmuls in 4 PE
           col-groups (~0.25us PE per head).
           Output is UNNORMALIZED bf16 out^T + per-head denominator
           rows; softmax division + v-path bias happen on host.
"""
import sys
import numpy as np

sys.path.insert(0, "/opt/trn_rl_repo")
import ml_dtypes  # noqa: E402

B, S, D = 2, 2048, 2048
G, HPG, HD = 4, 4, 128
GC = HPG * HD            # 512 columns per group
SCALE = HD ** -0.5
P = 128
KB = D // P              # 16 contraction blocks
SB = S // P              # 16 sk blocks
NCORES = 8

_CACHE: dict = {}

# early-attention config: head -> number of leading k-blocks whose
# qc<3 scores+exps run inside chunk 3's projection window
NE = {0: 12, 1: 10}
EQ = 1536                # early query columns (qc 0..2)


def _build_program():
    import concourse.tile_sem_assignment as tsa
    # Walrus caps sync waits per instruction; _split_excess_waits breaks
    # any multi-wait compute instruction into standalone EventSemaphore
    # CTRLs on the same engine.  Keep the default 8 HWDGE semaphores so
    # DMA-completion waits stay fine-grained.
    tsa.NUM_HWDGE_SEMS = 8

    import concourse.bass as bass
    import concourse.tile as tile
    from concourse import mybir
    from contextlib import ExitStack

    bf16 = mybir.dt.bfloat16
    f32 = mybir.dt.float32

    nc = bass.Bass(trn_type="TRN2")
    SCH = 512                 # s-chunk width for projection phase
    NCH = S // SCH            # 4 chunks
    QCH = 1024                # sq chunk width for scores/exp
    xt_d = nc.dram_tensor("xt", [NCH, P, KB, SCH], bf16, kind="ExternalInput")
    wqk_d = nc.dram_tensor("wqk", [P, KB, 2 * GC], bf16, kind="ExternalInput")
    wvf_d = nc.dram_tensor("wvf", [P, KB, GC], bf16, kind="ExternalInput")
    b1_d = nc.dram_tensor("b1", [P, 8], f32, kind="ExternalInput")
    onesc_d = nc.dram_tensor("onesc", [P, 1], bf16, kind="ExternalInput")
    out_d = nc.dram_tensor("out", [GC, S], bf16, kind="ExternalOutput")
    den_d = nc.dram_tensor("den", [HPG, 4, SCH], f32, kind="ExternalOutput")

    Exp = mybir.ActivationFunctionType.Exp
    Add = mybir.AluOpType.add

    # early units (h, j, qc) in emission order: h0 fully, then h1
    early_units = [(h, j, qc) for h in (0, 1)
                   for qc in range(3) for j in range(NE[h])]

    with tile.TileContext(nc) as tc:
        with ExitStack() as octx:
            # ---- persistent tiles ----
            persist = octx.enter_context(tc.tile_pool(name="persist", bufs=1))
            k_sb = persist.tile([P, HPG, S], bf16)       # k^T per head
            q2_sb = persist.tile([P, HPG, S], bf16)      # q2^T per head
            v2_sb = persist.tile([P, SB, GC], bf16)      # v2 natural blocks
            b1_sb = persist.tile([P, 8], f32)
            ones_sb = persist.tile([P, 1], bf16)
            nc.sync.dma_start(b1_sb[:], b1_d[:])
            nc.sync.dma_start(ones_sb[:], onesc_d[:])
            # early exp tiles (qc 0..2 only; live chunk 3 -> their PV)
            pearly = octx.enter_context(tc.tile_pool(name="pearly", bufs=1))
            PET = {h: [pearly.tile([P, EQ], bf16, tag=f"pe{h}_{j}",
                                   name=f"pe{h}_{j}") for j in range(NE[h])]
                   for h in NE}
            # head-2 j=2,3 full score tiles, exp'd inside chunk 3's
            # v-window (their inputs are all evacuated by then)
            pfxA = octx.enter_context(tc.tile_pool(name="pfxA", bufs=1))
            VFWD = {j: pfxA.tile([P, S], bf16, tag=f"fx{j}", name=f"vf{j}")
                    for j in (2, 3, 4)}

            # ---------------- phase 1: projections ----------------
            with ExitStack() as ctx:
                wpool = ctx.enter_context(tc.tile_pool(name="w1", bufs=1))
                xpool = ctx.enter_context(tc.tile_pool(name="xT", bufs=2))
                tpool = ctx.enter_context(tc.tile_pool(name="tmp", bufs=1))
                wqk_sb = wpool.tile([P, KB, 2 * GC], bf16)
                wvf_sb = wpool.tile([P, KB, GC], bf16)

                def wslice(m, k):
                    return wqk_sb[:, k, m * P:(m + 1) * P]

                Ident = mybir.ActivationFunctionType.Identity

                def evac(ps, m, c, on_act=False):
                    """psum -> q2/k SBUF with bias add (DVE or ACT)."""
                    dst = q2_sb if m < 4 else k_sb
                    sl = dst[:, m % 4, c * SCH:(c + 1) * SCH]
                    if on_act:
                        nc.scalar.activation(sl, ps[:], Ident,
                                             bias=b1_sb[:, m:m + 1])
                    else:
                        nc.vector.tensor_scalar_add(sl, ps[:],
                                                    b1_sb[:, m:m + 1])

                # stage A: chunk 0, m=0..7 (q2', k), K-OUTER in groups of
                # 4 k-blocks with 8 live PSUM banks.
                # ONE phase-1 psum pool with per-bank tags: stage B tiles
                # reuse stage A's banks tag-by-tag, so their WAR waits
                # only on that single bank's evacuation instead of on a
                # fresh-pool barrier over all 8 (a ~4us PE bubble).
                ph1ps = ctx.enter_context(
                    tc.tile_pool(name="ph1ps", bufs=1, space="PSUM"))
                xT0 = xpool.tile([P, KB, SCH], bf16, tag="xT")
                if True:
                    psA = [ph1ps.tile([P, SCH], f32, tag=f"b{m}",
                                      name=f"psA{m}")
                           for m in range(8)]
                    # warm the PE during the initial DMA window: dummy
                    # matmuls keep the HAM clock gate at 8/8 so the first
                    # real matmuls run at 2.4GHz.  17 matmuls ~ 5us,
                    # matching the arrival of the first k-block's data.
                    ws = tpool.tile([P, SCH], bf16, tag="ws", name="ws")
                    nc.any.memset(ws[:], 0.0)
                    for _ in range(15):
                        nc.tensor.matmul(psA[0][:], ws[:, 0:P], ws[:],
                                         start=True, stop=True)
                    # first k-block split across 4 queues so the first
                    # real matmul's ~0.4MB lands as early as possible
                    for qq in range(4):
                        sl = slice(qq * 256, qq * 256 + 256)
                        nc.sync.dma_start(wqk_sb[:, 0, sl], wqk_d[:, 0, sl])
                    nc.sync.dma_start(xT0[:, 0, 0:256], xt_d[0, :, 0, 0:256])
                    nc.sync.dma_start(xT0[:, 0, 256:512],
                                      xt_d[0, :, 0, 256:512])
                    kgroups = [(1, 2), (2, 3), (3, 4), (4, 6), (6, 8),
                               (8, 12), (12, 16)]
                    for lo, hi in kgroups:
                        ks = slice(lo, hi)
                        nc.sync.dma_start(wqk_sb[:, ks], wqk_d[:, ks])
                        nc.sync.dma_start(xT0[:, ks], xt_d[0, :, ks])
                    # 4 pieces on separate queues: one 2MB DMA on a
                    # single engine (~40us) would stall the chunk-0
                    # v-blocks by ~4us.
                    for g4 in range(4):
                        ks = slice(4 * g4, 4 * g4 + 4)
                        nc.sync.dma_start(wvf_sb[:, ks], wvf_d[:, ks])
                    for k in range(KB):
                        for m in range(8):
                            nc.tensor.matmul(
                                psA[m][:], wslice(m, k), xT0[:, k],
                                start=(k == 0), stop=(k == KB - 1))
                    for m in range(8):
                        # alternate DVE/ACT so the 8 serialized copies
                        # drain in ~3us instead of ~6us
                        evac(psA[m], m, 0, on_act=(m % 2 == 1))

                # stage B: v2 chunk 0, then all m for chunks 1..3, with
                # the early attention units woven into chunk 3's window.
                if True:
                    def vblock(xT, c, sb):
                        vt = ph1ps.tile([P, GC], f32, tag=f"b{sb % 2}",
                                        name="vt")
                        for k in range(KB):
                            nc.tensor.matmul(
                                vt[:], xT[:, k, sb * P:(sb + 1) * P],
                                wvf_sb[:, k, :],
                                start=(k == 0), stop=(k == KB - 1))
                        nc.vector.tensor_copy(
                            v2_sb[:, c * (SCH // P) + sb, :], vt[:])

                    eu_cnt = [0]

                    def escore(h, j, qc, dst):
                        eu_cnt[0] += 1
                        ss = ph1ps.tile([P, SCH], f32,
                                        tag=f"b{5 + eu_cnt[0] % 2}",
                                        name="ess")
                        nc.tensor.matmul(
                            ss[:], k_sb[:, h, j * P:(j + 1) * P],
                            q2_sb[:, h, qc * SCH:(qc + 1) * SCH],
                            start=True, stop=True)
                        nc.scalar.activation(
                            dst[:, qc * SCH:(qc + 1) * SCH], ss[:], Exp)

                    def early_unit(h, j, qc):
                        escore(h, j, qc, PET[h][j])

                    for sb in range(SCH // P):
                        vblock(xT0, 0, sb)
                    for c in range(1, NCH):
                        xT = xpool.tile([P, KB, SCH], bf16, tag="xT")
                        # 4 k-range pieces on separate queues
                        for g4 in range(4):
                            ks = slice(4 * g4, 4 * g4 + 4)
                            nc.sync.dma_start(xT[:, ks], xt_d[c, :, ks])
                        ew = list(early_units) if c == NCH - 1 else []
                        for m in range(8):
                            ps = ph1ps.tile([P, SCH], f32,
                                            tag=f"b{2 + m % 3}", name="pp")
                            for k in range(KB):
                                nc.tensor.matmul(
                                    ps[:], wslice(m, k), xT[:, k],
                                    start=(k == 0), stop=(k == KB - 1))
                            evac(ps, m, c)
                            for _ in range(6):
                                if ew:
                                    early_unit(*ew.pop(0))
                        vf = [(j, qc) for j in (2, 3, 4)
                              for qc in range(4)] if c == NCH - 1 else []
                        for sb in range(SCH // P):
                            vblock(xT, c, sb)
                            for _ in range(5):
                                if ew:
                                    early_unit(*ew.pop(0))
                            for _ in range(3):
                                if vf:
                                    j, qc = vf.pop(0)
                                    escore(2, j, qc, VFWD[j])
                        while ew:
                            early_unit(*ew.pop(0))
                        while vf:
                            j, qc = vf.pop(0)
                            escore(2, j, qc, VFWD[j])

            # ---------------- phase 2: attention ----------------
            with ExitStack() as ctx:
                ppool = ctx.enter_context(tc.tile_pool(name="P", bufs=10))
                pfx = ctx.enter_context(tc.tile_pool(name="pfx", bufs=1))
                q3pool = ctx.enter_context(tc.tile_pool(name="q3", bufs=11))
                opool = ctx.enter_context(tc.tile_pool(name="osb", bufs=4))
                dpool = ctx.enter_context(tc.tile_pool(name="dsb", bufs=1))
                dapool = ctx.enter_context(tc.tile_pool(name="dacc", bufs=1))
                sps = ctx.enter_context(
                    tc.tile_pool(name="sps", bufs=2, space="PSUM"))
                ops = ctx.enter_context(
                    tc.tile_pool(name="ops", bufs=1, space="PSUM"))

                # slot list: for early (h,j) only the qc3 scores remain.
                # Within a head, interleave the ACT-heavy full slots among
                # the PE-heavy q3 slots so neither engine stalls.
                slots = []
                for h in range(HPG):
                    ne = NE.get(h, 0)
                    q3l = [(h, j, True) for j in range(ne)]
                    ful = [(h, j, False) for j in range(ne, SB)]
                    nf, nq = len(ful), len(q3l)
                    fi = qi = 0
                    for t in range(nf + nq):
                        if qi >= nq or (fi < nf
                                        and fi * (nf + nq) <= t * nf):
                            slots.append(ful[fi])
                            fi += 1
                        else:
                            slots.append(q3l[qi])
                            qi += 1
                # the h0/h1 region is PE-bound (PV-heavy, exps mostly done
                # early) while h2/h3's is ACT-bound: forward 3 of h2's
                # score slots (exp-heavy; their PV stays in h2's window)
                # into the early region to balance both.
                FWD = [(2, j, False) for j in range(2)]
                for s in FWD:
                    slots.remove(s)
                for pos, s in zip((18, 8), FWD[::-1]):
                    slots.insert(pos, s)
                # j=2..4 were scored+exp'd in chunk 3's v-window
                for j in (2, 3, 4):
                    slots.remove((2, j, False))

                Q3T = {}          # (h,j) -> [P,512] qc3 exp tile
                PF = {}           # (h,j) -> [P,S] full exp tile
                dacc = {}         # h -> [P,S] bf16 denominator accum
                po = {}           # h -> 4 live PV psum banks
                ready = {}        # (h,j) -> slot idx of its last exp
                for j in (2, 3, 4):
                    PF[(2, j)] = VFWD[j]
                    ready[(2, j)] = -10 ** 9

                def pv_group(h, j):
                    early = j < NE.get(h, 0)
                    for qc in range(4):
                        if early and qc < 3:
                            rhs = PET[h][j][:, qc * 512:(qc + 1) * 512]
                        elif early:
                            rhs = Q3T[(h, j)][:]
                        else:
                            rhs = PF[(h, j)][:, qc * 512:(qc + 1) * 512]
                        nc.tensor.matmul(
                            po[h][qc][:], v2_sb[:, j, h * HD:(h + 1) * HD],
                            rhs, start=(j == 0), stop=(j == SB - 1))

                Ident2 = mybir.ActivationFunctionType.Identity

                def pv_den(h):
                    """PV epilogue + denominators for head h."""
                    for qc in range(4):
                        sl = slice(qc * 512, (qc + 1) * 512)
                        osb = opool.tile([P, 512], bf16, tag="o", name="osb")
                        # alternate DVE/ACT so the 4 serialized copies
                        # drain twice as fast (matters for the last head)
                        if qc % 2 == 0:
                            nc.vector.tensor_copy(osb[:], po[h][qc][:])
                        else:
                            nc.scalar.activation(osb[:], po[h][qc][:],
                                                 Ident2)
                        nc.sync.dma_start(out_d[h * P:(h + 1) * P, sl], osb[:])
                    # dacc (bf16) column-summed by 4 CONCURRENT M=1
                    # ones-matmuls in 4 PE col-groups
                    pd = ops.tile([P, 512], f32, tag="po0", name="pd")
                    for qc in range(4):
                        nc.tensor.matmul(
                            pd[32 * qc:32 * qc + 1, :],
                            ones_sb[:, 0:1],
                            dacc[h][:, qc * 512:(qc + 1) * 512],
                            start=True, stop=True,
                            tile_position=(0, 32 * qc))
                    d4 = dpool.tile([P, 512], f32, tag="dp", name="d4")
                    nc.vector.tensor_copy(d4[:], pd[:])
                    # single strided DMA: partitions {0,32,64,96} -> den[h]
                    d4v = d4[:].rearrange("(a b) f -> a b f", b=32)[:, 0:1, :]
                    nc.sync.dma_start(den_d[h], d4v)

                pvq = [(h, j) for h in range(HPG) for j in range(SB)]
                pvi = [0]
                LAG2 = 2

                def drain_pv(si, budget):
                    while pvi[0] < len(pvq) and budget > 0:
                        hp, jp = pvq[pvi[0]]
                        if ready.get((hp, jp), 10 ** 9) > si - LAG2:
                            return
                        if jp == 0:
                            po[hp] = [ops.tile([P, 512], f32, tag=f"po{qc}",
                                               name=f"po{qc}")
                                      for qc in range(4)]
                        pv_group(hp, jp)
                        if jp == SB - 1:
                            pv_den(hp)
                        pvi[0] += 1
                        budget -= 1

                for si, (h, j, is_q3) in enumerate(slots):
                    drain_pv(si, 2)
                    if (h, j) == (2, 5):
                        # fold the v-window-scored j=2..4 into h2's dacc
                        for jj in (2, 3, 4):
                            nc.vector.tensor_tensor(
                                dacc[2][:], dacc[2][:], VFWD[jj][:], op=Add)
                    first = h not in dacc
                    if first:
                        dacc[h] = dapool.tile([P, S], bf16, tag=f"da{h}",
                                              name=f"da{h}")
                    if is_q3:
                        ss = sps.tile([P, QCH], f32, name="ss")
                        nc.tensor.matmul(
                            ss[:, 0:512], k_sb[:, h, j * P:(j + 1) * P],
                            q2_sb[:, h, EQ:S], start=True, stop=True)
                        q3t = q3pool.tile([P, 512], bf16, tag="q3",
                                          name="q3t")
                        nc.scalar.activation(q3t[:], ss[:, 0:512], Exp)
                        Q3T[(h, j)] = q3t
                        # denominator accumulation (DVE, bf16 2x rate)
                        if first:
                            nc.vector.tensor_copy(
                                dacc[h][:, 0:EQ], PET[h][j][:])
                            nc.vector.tensor_copy(
                                dacc[h][:, EQ:S], q3t[:])
                        else:
                            nc.vector.tensor_tensor(
                                dacc[h][:, 0:EQ], dacc[h][:, 0:EQ],
                                PET[h][j][:], op=Add)
                            nc.vector.tensor_tensor(
                                dacc[h][:, EQ:S], dacc[h][:, EQ:S],
                                q3t[:], op=Add)
                    else:
                        if (h, j, False) in FWD:
                            # forwarded slots live much longer than the
                            # ppool rotation: dedicated buffers
                            Pj = pfx.tile([P, S], bf16, tag=f"fx{j}",
                                          name="Pfx")
                        else:
                            Pj = ppool.tile([P, S], bf16, tag="P", name="Pj")
                        for qc in range(S // QCH):
                            ss = sps.tile([P, QCH], f32, name="ss")
                            for half in range(QCH // 512):
                                off = qc * QCH + half * 512
                                nc.tensor.matmul(
                                    ss[:, half * 512:(half + 1) * 512],
                                    k_sb[:, h, j * P:(j + 1) * P],
                                    q2_sb[:, h, off:off + 512],
                                    start=True, stop=True)
                            nc.scalar.activation(
                                Pj[:, qc * QCH:(qc + 1) * QCH], ss[:], Exp)
                        PF[(h, j)] = Pj
                        if first:
                            nc.vector.tensor_copy(dacc[h][:], Pj[:])
                        elif si == len(slots) - 1:
                            # last slot: slice the add per query-chunk so
                            # the concurrent pd col-group matmuls can
                            # start before the full add finishes
                            for qc in range(4):
                                sl = slice(qc * 512, (qc + 1) * 512)
                                nc.vector.tensor_tensor(
                                    dacc[h][:, sl], dacc[h][:, sl],
                                    Pj[:, sl], op=Add)
                        else:
                            nc.vector.tensor_tensor(
                                dacc[h][:], dacc[h][:], Pj[:], op=Add)
                    ready[(h, j)] = si
                    drain_pv(si, 1)
                drain_pv(10 ** 9, 10 ** 9)

    _split_excess_waits(nc, mybir)
    return nc


def _split_excess_waits(nc, mybir):
    """Each TPB instruction has ONE wait slot (NEURON_ISA_TPB_EVENTS); walrus
    refuses instructions with more sync waits.  Tile attaches the full
    vector-clock wait list to instructions, so split all but one wait out
    into standalone EventSemaphore (CTRL) instructions on the same engine,
    placed immediately before.  Semantics are identical: all waits must be
    satisfied before the instruction executes."""
    import copy
    template = None
    for blk in nc.m.functions[0].blocks:
        for inst in blk.instructions:
            if isinstance(inst, mybir.InstEventSemaphore):
                template = inst
                break
        if template is not None:
            break
    assert template is not None, "no EventSemaphore template found"
    uid = [0]
    for fn in nc.m.functions:
        for blk in fn.blocks:
            out = []
            for inst in blk.instructions:
                si = inst.sync_info
                if si is not None and len(si.on_wait) > 1:
                    waits = list(si.on_wait)
                    for w in waits[:-1]:
                        ev = copy.deepcopy(template)
                        ev.name = f"swsplit-{uid[0]}"
                        uid[0] += 1
                        ev.engine = inst.engine
                        ev.sync_info = mybir.SyncInfo(on_wait=[w], on_update=[])
                        out.append(ev)
                    si.on_wait = waits[-1:]
                    inst.sync_info = si
                out.append(inst)
            blk.instructions[:] = out
    return nc


def _numpy_fallback(x, attn_mask, Wqkv, bqkv, Wq, bq, Wv, bv):
    x = np.asarray(x, np.float32)
    qkv = x @ np.asarray(Wqkv, np.float32) + np.asarray(bqkv, np.float32)
    q, k, v = np.split(qkv, 3, axis=-1)
    q = q.reshape(B, S, G, HPG, HD)
    k = k.reshape(B, S, G, HPG, HD)
    v = v.reshape(B, S, G, HPG, HD)
    q = np.einsum('bsghd,gde->bsghe', q, np.asarray(Wq, np.float32)) \
        + np.asarray(bq, np.float32)[None, None, :, None, :]
    v = np.einsum('bsghd,gde->bsghe', v, np.asarray(Wv, np.float32)) \
        + np.asarray(bv, np.float32)[None, None, :, None, :]
    out = np.empty((B, S, G, HPG, HD), np.float32)
    for b in range(B):
        for g in range(G):
            for hh in range(HPG):
                s = (q[b, :, g, hh] @ k[b, :, g, hh].T) * SCALE
                s = s - s.max(axis=-1, keepdims=True)
                p = np.exp(s)
                p /= p.sum(axis=-1, keepdims=True)
                p = p * np.asarray(attn_mask, np.float32)
                out[b, :, g, hh] = p @ v[b, :, g, hh]
    return out.reshape(B, S, D)


def kernel(x, attn_mask, Wqkv, bqkv, Wq, bq, Wv, bv):
    x = np.asarray(x)
    attn_mask = np.asarray(attn_mask)
    Wqkv = np.asarray(Wqkv)
    bqkv = np.asarray(bqkv)
    Wq = np.asarray(Wq)
    bq = np.asarray(bq)
    Wv = np.asarray(Wv)
    bv = np.asarray(bv)

    if not np.all(attn_mask == 1.0):
        # general (non-ones) post-softmax mask: correct but slow host path
        return _numpy_fallback(x, attn_mask, Wqkv, bqkv, Wq, bq, Wv, bv)

    if "nc" not in _CACHE:
        _CACHE["nc"] = _build_program()
    nc = _CACHE["nc"]
    from concourse.bass_utils import run_bass_kernel_spmd

    bf = ml_dtypes.bfloat16
    in_maps = []
    # xt layout [chunk, p, ko, s']: xt[c,p,ko,s'] = x[b][c*512+s', ko*128+p]
    x_bf = []
    for b in range(B):
        xT = np.asarray(x[b], np.float32).T.astype(bf)      # [D, S]
        x_bf.append(np.ascontiguousarray(
            xT.reshape(KB, P, NCORES // 2, 512).transpose(2, 1, 0, 3)))
    Wq32 = np.asarray(Wq, np.float32)
    Wv32 = np.asarray(Wv, np.float32)
    host_bias = []

    def pmajor(w):
        """[D, N] -> [P, KB, N] with [p, ko, n] = w[ko*128+p, n]"""
        return np.ascontiguousarray(
            w.reshape(KB, P, w.shape[1]).transpose(1, 0, 2))

    for c in range(NCORES):
        b, g = divmod(c, G)
        cols = slice(g * GC, (g + 1) * GC)
        wq_c = Wqkv[:, 0 * D:1 * D][:, cols].astype(np.float32)
        wk_c = Wqkv[:, 1 * D:2 * D][:, cols]
        wv_c = Wqkv[:, 2 * D:3 * D][:, cols].astype(np.float32)
        # fold the per-group query/value projections (+ attention scale)
        # on host:
        wqs = Wq32[g] * SCALE
        wq_fold = (wq_c.reshape(D, HPG, HD) @ wqs[None]).reshape(D, GC)
        wv_fold = (wv_c.reshape(D, HPG, HD) @ Wv32[g][None]).reshape(D, GC)
        wqk = np.concatenate([wq_fold.astype(bf), np.asarray(wk_c, bf)],
                             axis=1)
        bq1 = bqkv[0 * D:1 * D][cols].astype(np.float32)
        bk1 = bqkv[1 * D:2 * D][cols].astype(np.float32)
        bv1 = bqkv[2 * D:3 * D][cols].astype(np.float32)
        bq2 = (bq1.reshape(HPG, HD) @ wqs
               + np.asarray(bq, np.float32)[g] * SCALE).reshape(GC)
        b1cat = np.concatenate([bq2, bk1]).astype(np.float32)
        host_bias.append(
            (bv1.reshape(HPG, HD) @ Wv32[g]
             + np.asarray(bv, np.float32)[g][None, :]).reshape(GC))
        in_maps.append({
            "xt": x_bf[b],
            "wqk": pmajor(wqk),
            "wvf": pmajor(wv_fold.astype(bf)),
            "b1": np.ascontiguousarray(b1cat.reshape(8, P).T),
            "onesc": np.ones((P, 1), bf),
        })

    res = run_bass_kernel_spmd(nc, in_maps, list(range(NCORES)),
                               **_CACHE.get("run_kwargs", {}))
    _CACHE["last_results"] = res

    out = np.empty((B, S, D), np.float32)
    for c in range(NCORES):
        b, g = divmod(c, G)
        o = np.asarray(res.results[c]["out"], np.float32)  # [GC,S] out^T
        den = res.results[c]["den"].reshape(HPG, S)        # [HPG,4,512]
        o = o / np.repeat(den, HD, axis=0)  # normalize rows h*128+e by den[h]
        o = o + host_bias[c][:, None]
        out[b, :, g * GC:(g + 1) * GC] = o.T
    return out


# revision 69
# speedup vs baseline: 1.0140x; 1.0140x over previous
"""GQA dense-transformer kernel for 8 Trainium2 NeuronCores.

Problem (hardcoded): B=2, S=2048, D=2048, kv_heads=16, groups G=4, HPG=4,
HD=128.  reference:
    qkv = x @ Wqkv + bqkv ; q,k,v = split(qkv)
    q = einsum('bsghd,gde->bsghe', q, Wq) + bq   (per-group shared proj)
    v = einsum('bsghd,gde->bsghe', v, Wv) + bv
    scores = einsum('bqghd,bkghd->bghqk', q, k) / sqrt(HD)
    attn = softmax(scores) * attn_mask           (mask == ones at grading)
    out = einsum('bghqk,bkghd->bqghd', attn, v)  -> [B,S,D]

Sharding: core c = b*4 + g handles (batch b, group g): it computes the
512 output columns [g*512,(g+1)*512) of out[b].

Per-core device program (bf16 matmuls, fp32 PSUM):
  BOTH per-group projections are folded into Wqkv on the HOST:
    W_q2'[:,hs] = Wqkv_q[:,hs] @ (Wq[g]*SCALE)   (query path + attn scale)
    W_v2'[:,hs] = Wqkv_v[:,hs] @ Wv[g]           (value path)
  so the device computes q2^T, k^T and v2 directly in one projection
  pass.  Inputs are RELAID OUT partition-major on the host so every DMA
  descriptor covers a 1-16KB contiguous run.
  phase 1: chunk 0 (s in [0,512)) runs K-OUTER in groups of 4 k-blocks
           with 8 live PSUM banks (m=0..7 = q2',k); v2 and chunks 1..3
           run m-outer.  v2 is computed in NATURAL (key-partition)
           layout directly: per 128-key block, lhsT = the xT key slice
           (stationary), rhs = W_v2' k-slab, PSUM-accumulated over the
           16 k-blocks.  q2/k evacuate on DVE (tensor_scalar bias add),
           v2 on DVE, keeping ACT free for early attention.
  early attention: scores+exp for heads 0/1, k-blocks j<12 (chunks
           0..2), query columns [0,1536) are INTERLEAVED into chunk 3's
           projection window, so the ACT engine (the phase-2
           bottleneck) starts ~60us early and phase 2 shrinks.
  phase 2: a single SLOT SCHEDULER walks (head, j) score slots -- for
           early (h,j) only the last query chunk remains -- and weaves
           the global PV queue (4 col-chunk matmuls per (h,j), j-MAJOR
           PSUM accumulation, 2-slot lag behind the exps) plus the
           denominator accumulation into the stream, so the PE and DVE
           track ACT with no per-head barrier.
           Denominators: dacc[h] += P_j on the DVE (bf16, 2x rate),
           column-summed by 4 CONCURRENT M=1 ones-matmuls in 4 PE
           col-groups (~0.25us PE per head).
           Output is UNNORMALIZED bf16 out^T + per-head denominator
           rows; softmax division + v-path bias happen on host.
"""
import sys
import numpy as np

sys.path.insert(0, "/opt/trn_rl_repo")
import ml_dtypes  # noqa: E402

B, S, D = 2, 2048, 2048
G, HPG, HD = 4, 4, 128
GC = HPG * HD            # 512 columns per group
SCALE = HD ** -0.5
P = 128
KB = D // P              # 16 contraction blocks
SB = S // P              # 16 sk blocks
NCORES = 8

_CACHE: dict = {}

# early-attention config: head -> number of leading k-blocks whose
# qc<3 scores+exps run inside chunk 3's projection window
NE = {0: 12, 1: 10}
EQ = 1536                # early query columns (qc 0..2)


def _build_program():
    import concourse.tile_sem_assignment as tsa
    # Walrus caps sync waits per instruction; _split_excess_waits breaks
    # any multi-wait compute instruction into standalone EventSemaphore
    # CTRLs on the same engine.  Keep the default 8 HWDGE semaphores so
    # DMA-completion waits stay fine-grained.
    tsa.NUM_HWDGE_SEMS = 8

    import concourse.bass as bass
    import concourse.tile as tile
    from concourse import mybir
    from contextlib import ExitStack

    bf16 = mybir.dt.bfloat16
    f32 = mybir.dt.float32

    nc = bass.Bass(trn_type="TRN2")
    SCH = 512                 # s-chunk width for projection phase
    NCH = S // SCH            # 4 chunks
    QCH = 1024                # sq chunk width for scores/exp
    xt_d = nc.dram_tensor("xt", [NCH, P, KB, SCH], bf16, kind="ExternalInput")
    wqk_d = nc.dram_tensor("wqk", [P, KB, 2 * GC], bf16, kind="ExternalInput")
    wvf_d = nc.dram_tensor("wvf", [P, KB, GC], bf16, kind="ExternalInput")
    b1_d = nc.dram_tensor("b1", [P, 8], f32, kind="ExternalInput")
    onesc_d = nc.dram_tensor("onesc", [P, 1], bf16, kind="ExternalInput")
    out_d = nc.dram_tensor("out", [GC, S], bf16, kind="ExternalOutput")
    den_d = nc.dram_tensor("den", [HPG, 4, SCH], f32, kind="ExternalOutput")

    Exp = mybir.ActivationFunctionType.Exp
    Add = mybir.AluOpType.add

    # early units (h, j, qc) in emission order: h0 fully, then h1
    early_units = [(h, j, qc) for h in (0, 1)
                   for qc in range(3) for j in range(NE[h])]

    with tile.TileContext(nc) as tc:
        with ExitStack() as octx:
            # ---- persistent tiles ----
            persist = octx.enter_context(tc.tile_pool(name="persist", bufs=1))
            k_sb = persist.tile([P, HPG, S], bf16)       # k^T per head
            q2_sb = persist.tile([P, HPG, S], bf16)      # q2^T per head
            v2_sb = persist.tile([P, SB, GC], bf16)      # v2 natural blocks
            b1_sb = persist.tile([P, 8], f32)
            ones_sb = persist.tile([P, 1], bf16)
            nc.sync.dma_start(b1_sb[:], b1_d[:])
            nc.sync.dma_start(ones_sb[:], onesc_d[:])
            # early exp tiles (qc 0..2 only; live chunk 3 -> their PV)
            pearly = octx.enter_context(tc.tile_pool(name="pearly", bufs=1))
            PET = {h: [pearly.tile([P, EQ], bf16, tag=f"pe{h}_{j}",
                                   name=f"pe{h}_{j}") for j in range(NE[h])]
                   for h in NE}
            # head-2 j=2,3 full score tiles, exp'd inside chunk 3's
            # v-window (their inputs are all evacuated by then)
            pfxA = octx.enter_context(tc.tile_pool(name="pfxA", bufs=1))
            VFWD = {j: pfxA.tile([P, S], bf16, tag=f"fx{j}", name=f"vf{j}")
                    for j in (2, 3, 4)}

            # ---------------- phase 1: projections ----------------
            with ExitStack() as ctx:
                wpool = ctx.enter_context(tc.tile_pool(name="w1", bufs=1))
                xpool = ctx.enter_context(tc.tile_pool(name="xT", bufs=2))
                tpool = ctx.enter_context(tc.tile_pool(name="tmp", bufs=1))
                wqk_sb = wpool.tile([P, KB, 2 * GC], bf16)
                wvf_sb = wpool.tile([P, KB, GC], bf16)

                def wslice(m, k):
                    return wqk_sb[:, k, m * P:(m + 1) * P]

                Ident = mybir.ActivationFunctionType.Identity

                def evac(ps, m, c, on_act=False):
                    """psum -> q2/k SBUF with bias add (DVE or ACT)."""
                    dst = q2_sb if m < 4 else k_sb
                    sl = dst[:, m % 4, c * SCH:(c + 1) * SCH]
                    if on_act:
                        nc.scalar.activation(sl, ps[:], Ident,
                                             bias=b1_sb[:, m:m + 1])
                    else:
                        nc.vector.tensor_scalar_add(sl, ps[:],
                                                    b1_sb[:, m:m + 1])

                # stage A: chunk 0, m=0..7 (q2', k), K-OUTER in groups of
                # 4 k-blocks with 8 live PSUM banks.
                # ONE phase-1 psum pool with per-bank tags: stage B tiles
                # reuse stage A's banks tag-by-tag, so their WAR waits
                # only on that single bank's evacuation instead of on a
                # fresh-pool barrier over all 8 (a ~4us PE bubble).
                ph1ps = ctx.enter_context(
                    tc.tile_pool(name="ph1ps", bufs=1, space="PSUM"))
                xT0 = xpool.tile([P, KB, SCH], bf16, tag="xT")
                if True:
                    psA = [ph1ps.tile([P, SCH], f32, tag=f"b{m}",
                                      name=f"psA{m}")
                           for m in range(8)]
                    # warm the PE during the initial DMA window: dummy
                    # matmuls keep the HAM clock gate at 8/8 so the first
                    # real matmuls run at 2.4GHz.  17 matmuls ~ 5us,
                    # matching the arrival of the first k-block's data.
                    ws = tpool.tile([P, SCH], bf16, tag="ws", name="ws")
                    nc.any.memset(ws[:], 0.0)
                    for _ in range(17):
                        nc.tensor.matmul(psA[0][:], ws[:, 0:P], ws[:],
                                         start=True, stop=True)
                    # first k-block split across 4 queues so the first
                    # real matmul's ~0.4MB lands as early as possible
                    nc.sync.dma_start(wqk_sb[:, 0, 0:512], wqk_d[:, 0, 0:512])
                    nc.sync.dma_start(wqk_sb[:, 0, 512:1024],
                                      wqk_d[:, 0, 512:1024])
                    nc.sync.dma_start(xT0[:, 0], xt_d[0, :, 0])
                    kgroups = [(1, 2), (2, 3), (3, 4), (4, 6), (6, 8),
                               (8, 12), (12, 16)]
                    for lo, hi in kgroups:
                        ks = slice(lo, hi)
                        nc.sync.dma_start(wqk_sb[:, ks], wqk_d[:, ks])
                        nc.sync.dma_start(xT0[:, ks], xt_d[0, :, ks])
                    # 4 pieces on separate queues: one 2MB DMA on a
                    # single engine (~40us) would stall the chunk-0
                    # v-blocks by ~4us.
                    for g4 in range(4):
                        ks = slice(4 * g4, 4 * g4 + 4)
                        nc.sync.dma_start(wvf_sb[:, ks], wvf_d[:, ks])
                    for k in range(KB):
                        for m in range(8):
                            nc.tensor.matmul(
                                psA[m][:], wslice(m, k), xT0[:, k],
                                start=(k == 0), stop=(k == KB - 1))
                    for m in range(8):
                        # alternate DVE/ACT so the 8 serialized copies
                        # drain in ~3us instead of ~6us
                        evac(psA[m], m, 0, on_act=(m % 2 == 1))

                # stage B: v2 chunk 0, then all m for chunks 1..3, with
                # the early attention units woven into chunk 3's window.
                if True:
                    def vblock(xT, c, sb):
                        vt = ph1ps.tile([P, GC], f32, tag=f"b{sb % 2}",
                                        name="vt")
                        for k in range(KB):
                            nc.tensor.matmul(
                                vt[:], xT[:, k, sb * P:(sb + 1) * P],
                                wvf_sb[:, k, :],
                                start=(k == 0), stop=(k == KB - 1))
                        nc.vector.tensor_copy(
                            v2_sb[:, c * (SCH // P) + sb, :], vt[:])

                    eu_cnt = [0]

                    def escore(h, j, qc, dst):
                        eu_cnt[0] += 1
                        ss = ph1ps.tile([P, SCH], f32,
                                        tag=f"b{5 + eu_cnt[0] % 2}",
                                        name="ess")
                        nc.tensor.matmul(
                            ss[:], k_sb[:, h, j * P:(j + 1) * P],
                            q2_sb[:, h, qc * SCH:(qc + 1) * SCH],
                            start=True, stop=True)
                        nc.scalar.activation(
                            dst[:, qc * SCH:(qc + 1) * SCH], ss[:], Exp)

                    def early_unit(h, j, qc):
                        escore(h, j, qc, PET[h][j])

                    for sb in range(SCH // P):
                        vblock(xT0, 0, sb)
                    for c in range(1, NCH):
                        xT = xpool.tile([P, KB, SCH], bf16, tag="xT")
                        # 4 k-range pieces on separate queues
                        for g4 in range(4):
                            ks = slice(4 * g4, 4 * g4 + 4)
                            nc.sync.dma_start(xT[:, ks], xt_d[c, :, ks])
                        ew = list(early_units) if c == NCH - 1 else []
                        for m in range(8):
                            ps = ph1ps.tile([P, SCH], f32,
                                            tag=f"b{2 + m % 3}", name="pp")
                            for k in range(KB):
                                nc.tensor.matmul(
                                    ps[:], wslice(m, k), xT[:, k],
                                    start=(k == 0), stop=(k == KB - 1))
                            evac(ps, m, c)
                            for _ in range(6):
                                if ew:
                                    early_unit(*ew.pop(0))
                        vf = [(j, qc) for j in (2, 3, 4)
                              for qc in range(4)] if c == NCH - 1 else []
                        for sb in range(SCH // P):
                            vblock(xT, c, sb)
                            for _ in range(5):
                                if ew:
                                    early_unit(*ew.pop(0))
                            for _ in range(3):
                                if vf:
                                    j, qc = vf.pop(0)
                                    escore(2, j, qc, VFWD[j])
                        while ew:
                            early_unit(*ew.pop(0))
                        while vf:
                            j, qc = vf.pop(0)
                            escore(2, j, qc, VFWD[j])

            # ---------------- phase 2: attention ----------------
            with ExitStack() as ctx:
                ppool = ctx.enter_context(tc.tile_pool(name="P", bufs=10))
                pfx = ctx.enter_context(tc.tile_pool(name="pfx", bufs=1))
                q3pool = ctx.enter_context(tc.tile_pool(name="q3", bufs=11))
                opool = ctx.enter_context(tc.tile_pool(name="osb", bufs=4))
                dpool = ctx.enter_context(tc.tile_pool(name="dsb", bufs=1))
                dapool = ctx.enter_context(tc.tile_pool(name="dacc", bufs=1))
                sps = ctx.enter_context(
                    tc.tile_pool(name="sps", bufs=2, space="PSUM"))
                ops = ctx.enter_context(
                    tc.tile_pool(name="ops", bufs=1, space="PSUM"))

                # slot list: for early (h,j) only the qc3 scores remain.
                # Within a head, interleave the ACT-heavy full slots among
                # the PE-heavy q3 slots so neither engine stalls.
                slots = []
                for h in range(HPG):
                    ne = NE.get(h, 0)
                    q3l = [(h, j, True) for j in range(ne)]
                    ful = [(h, j, False) for j in range(ne, SB)]
                    nf, nq = len(ful), len(q3l)
                    fi = qi = 0
                    for t in range(nf + nq):
                        if qi >= nq or (fi < nf
                                        and fi * (nf + nq) <= t * nf):
                            slots.append(ful[fi])
                            fi += 1
                        else:
                            slots.append(q3l[qi])
                            qi += 1
                # the h0/h1 region is PE-bound (PV-heavy, exps mostly done
                # early) while h2/h3's is ACT-bound: forward 3 of h2's
                # score slots (exp-heavy; their PV stays in h2's window)
                # into the early region to balance both.
                FWD = [(2, j, False) for j in range(2)]
                for s in FWD:
                    slots.remove(s)
                for pos, s in zip((18, 8), FWD[::-1]):
                    slots.insert(pos, s)
                # j=2..4 were scored+exp'd in chunk 3's v-window
                for j in (2, 3, 4):
                    slots.remove((2, j, False))

                Q3T = {}          # (h,j) -> [P,512] qc3 exp tile
                PF = {}           # (h,j) -> [P,S] full exp tile
                dacc = {}         # h -> [P,S] bf16 denominator accum
                po = {}           # h -> 4 live PV psum banks
                ready = {}        # (h,j) -> slot idx of its last exp
                for j in (2, 3, 4):
                    PF[(2, j)] = VFWD[j]
                    ready[(2, j)] = -10 ** 9

                def pv_group(h, j):
                    early = j < NE.get(h, 0)
                    for qc in range(4):
                        if early and qc < 3:
                            rhs = PET[h][j][:, qc * 512:(qc + 1) * 512]
                        elif early:
                            rhs = Q3T[(h, j)][:]
                        else:
                            rhs = PF[(h, j)][:, qc * 512:(qc + 1) * 512]
                        nc.tensor.matmul(
                            po[h][qc][:], v2_sb[:, j, h * HD:(h + 1) * HD],
                            rhs, start=(j == 0), stop=(j == SB - 1))

                Ident2 = mybir.ActivationFunctionType.Identity

                def pv_den(h):
                    """PV epilogue + denominators for head h."""
                    for qc in range(4):
                        sl = slice(qc * 512, (qc + 1) * 512)
                        osb = opool.tile([P, 512], bf16, tag="o", name="osb")
                        # alternate DVE/ACT so the 4 serialized copies
                        # drain twice as fast (matters for the last head)
                        if qc % 2 == 0:
                            nc.vector.tensor_copy(osb[:], po[h][qc][:])
                        else:
                            nc.scalar.activation(osb[:], po[h][qc][:],
                                                 Ident2)
                        nc.sync.dma_start(out_d[h * P:(h + 1) * P, sl], osb[:])
                    # dacc (bf16) column-summed by 4 CONCURRENT M=1
                    # ones-matmuls in 4 PE col-groups
                    pd = ops.tile([P, 512], f32, tag="po0", name="pd")
                    for qc in range(4):
                        nc.tensor.matmul(
                            pd[32 * qc:32 * qc + 1, :],
                            ones_sb[:, 0:1],
                            dacc[h][:, qc * 512:(qc + 1) * 512],
                            start=True, stop=True,
                            tile_position=(0, 32 * qc))
                    d4 = dpool.tile([P, 512], f32, tag="dp", name="d4")
                    nc.vector.tensor_copy(d4[:], pd[:])
                    # single strided DMA: partitions {0,32,64,96} -> den[h]
                    d4v = d4[:].rearrange("(a b) f -> a b f", b=32)[:, 0:1, :]
                    nc.sync.dma_start(den_d[h], d4v)

                pvq = [(h, j) for h in range(HPG) for j in range(SB)]
                pvi = [0]
                LAG2 = 2

                def drain_pv(si, budget):
                    while pvi[0] < len(pvq) and budget > 0:
                        hp, jp = pvq[pvi[0]]
                        if ready.get((hp, jp), 10 ** 9) > si - LAG2:
                            return
                        if jp == 0:
                            po[hp] = [ops.tile([P, 512], f32, tag=f"po{qc}",
                                               name=f"po{qc}")
                                      for qc in range(4)]
                        pv_group(hp, jp)
                        if jp == SB - 1:
                            pv_den(hp)
                        pvi[0] += 1
                        budget -= 1

                for si, (h, j, is_q3) in enumerate(slots):
                    drain_pv(si, 1)
                    if (h, j) == (2, 5):
                        # fold the v-window-scored j=2..4 into h2's dacc
                        for jj in (2, 3, 4):
                            nc.vector.tensor_tensor(
                                dacc[2][:], dacc[2][:], VFWD[jj][:], op=Add)
                    first = h not in dacc
                    if first:
                        dacc[h] = dapool.tile([P, S], bf16, tag=f"da{h}",
                                              name=f"da{h}")
                    if is_q3:
                        ss = sps.tile([P, QCH], f32, name="ss")
                        nc.tensor.matmul(
                            ss[:, 0:512], k_sb[:, h, j * P:(j + 1) * P],
                            q2_sb[:, h, EQ:S], start=True, stop=True)
                        q3t = q3pool.tile([P, 512], bf16, tag="q3",
                                          name="q3t")
                        nc.scalar.activation(q3t[:], ss[:, 0:512], Exp)
                        Q3T[(h, j)] = q3t
                        # denominator accumulation (DVE, bf16 2x rate)
                        if first:
                            nc.vector.tensor_copy(
                                dacc[h][:, 0:EQ], PET[h][j][:])
                            nc.vector.tensor_copy(
                                dacc[h][:, EQ:S], q3t[:])
                        else:
                            nc.vector.tensor_tensor(
                                dacc[h][:, 0:EQ], dacc[h][:, 0:EQ],
                                PET[h][j][:], op=Add)
                            nc.vector.tensor_tensor(
                                dacc[h][:, EQ:S], dacc[h][:, EQ:S],
                                q3t[:], op=Add)
                    else:
                        if (h, j, False) in FWD:
                            # forwarded slots live much longer than the
                            # ppool rotation: dedicated buffers
                            Pj = pfx.tile([P, S], bf16, tag=f"fx{j}",
                                          name="Pfx")
                        else:
                            Pj = ppool.tile([P, S], bf16, tag="P", name="Pj")
                        for qc in range(S // QCH):
                            ss = sps.tile([P, QCH], f32, name="ss")
                            for half in range(QCH // 512):
                                off = qc * QCH + half * 512
                                nc.tensor.matmul(
                                    ss[:, half * 512:(half + 1) * 512],
                                    k_sb[:, h, j * P:(j + 1) * P],
                                    q2_sb[:, h, off:off + 512],
                                    start=True, stop=True)
                            nc.scalar.activation(
                                Pj[:, qc * QCH:(qc + 1) * QCH], ss[:], Exp)
                        PF[(h, j)] = Pj
                        if first:
                            nc.vector.tensor_copy(dacc[h][:], Pj[:])
                        elif si == len(slots) - 1:
                            # last slot: slice the add per query-chunk so
                            # the concurrent pd col-group matmuls can
                            # start before the full add finishes
                            for qc in range(4):
                                sl = slice(qc * 512, (qc + 1) * 512)
                                nc.vector.tensor_tensor(
                                    dacc[h][:, sl], dacc[h][:, sl],
                                    Pj[:, sl], op=Add)
                        else:
                            nc.vector.tensor_tensor(
                                dacc[h][:], dacc[h][:], Pj[:], op=Add)
                    ready[(h, j)] = si
                    drain_pv(si, 1)
                drain_pv(10 ** 9, 10 ** 9)

    _split_excess_waits(nc, mybir)
    return nc


def _split_excess_waits(nc, mybir):
    """Each TPB instruction has ONE wait slot (NEURON_ISA_TPB_EVENTS); walrus
    refuses instructions with more sync waits.  Tile attaches the full
    vector-clock wait list to instructions, so split all but one wait out
    into standalone EventSemaphore (CTRL) instructions on the same engine,
    placed immediately before.  Semantics are identical: all waits must be
    satisfied before the instruction executes."""
    import copy
    template = None
    for blk in nc.m.functions[0].blocks:
        for inst in blk.instructions:
            if isinstance(inst, mybir.InstEventSemaphore):
                template = inst
                break
        if template is not None:
            break
    assert template is not None, "no EventSemaphore template found"
    uid = [0]
    for fn in nc.m.functions:
        for blk in fn.blocks:
            out = []
            for inst in blk.instructions:
                si = inst.sync_info
                if si is not None and len(si.on_wait) > 1:
                    waits = list(si.on_wait)
                    for w in waits[:-1]:
                        ev = copy.deepcopy(template)
                        ev.name = f"swsplit-{uid[0]}"
                        uid[0] += 1
                        ev.engine = inst.engine
                        ev.sync_info = mybir.SyncInfo(on_wait=[w], on_update=[])
                        out.append(ev)
                    si.on_wait = waits[-1:]
                    inst.sync_info = si
                out.append(inst)
            blk.instructions[:] = out
    return nc


def _numpy_fallback(x, attn_mask, Wqkv, bqkv, Wq, bq, Wv, bv):
    x = np.asarray(x, np.float32)
    qkv = x @ np.asarray(Wqkv, np.float32) + np.asarray(bqkv, np.float32)
    q, k, v = np.split(qkv, 3, axis=-1)
    q = q.reshape(B, S, G, HPG, HD)
    k = k.reshape(B, S, G, HPG, HD)
    v = v.reshape(B, S, G, HPG, HD)
    q = np.einsum('bsghd,gde->bsghe', q, np.asarray(Wq, np.float32)) \
        + np.asarray(bq, np.float32)[None, None, :, None, :]
    v = np.einsum('bsghd,gde->bsghe', v, np.asarray(Wv, np.float32)) \
        + np.asarray(bv, np.float32)[None, None, :, None, :]
    out = np.empty((B, S, G, HPG, HD), np.float32)
    for b in range(B):
        for g in range(G):
            for hh in range(HPG):
                s = (q[b, :, g, hh] @ k[b, :, g, hh].T) * SCALE
                s = s - s.max(axis=-1, keepdims=True)
                p = np.exp(s)
                p /= p.sum(axis=-1, keepdims=True)
                p = p * np.asarray(attn_mask, np.float32)
                out[b, :, g, hh] = p @ v[b, :, g, hh]
    return out.reshape(B, S, D)


def kernel(x, attn_mask, Wqkv, bqkv, Wq, bq, Wv, bv):
    x = np.asarray(x)
    attn_mask = np.asarray(attn_mask)
    Wqkv = np.asarray(Wqkv)
    bqkv = np.asarray(bqkv)
    Wq = np.asarray(Wq)
    bq = np.asarray(bq)
    Wv = np.asarray(Wv)
    bv = np.asarray(bv)

    if not np.all(attn_mask == 1.0):
        # general (non-ones) post-softmax mask: correct but slow host path
        return _numpy_fallback(x, attn_mask, Wqkv, bqkv, Wq, bq, Wv, bv)

    if "nc" not in _CACHE:
        _CACHE["nc"] = _build_program()
    nc = _CACHE["nc"]
    from concourse.bass_utils import run_bass_kernel_spmd

    bf = ml_dtypes.bfloat16
    in_maps = []
    # xt layout [chunk, p, ko, s']: xt[c,p,ko,s'] = x[b][c*512+s', ko*128+p]
    x_bf = []
    for b in range(B):
        xT = np.asarray(x[b], np.float32).T.astype(bf)      # [D, S]
        x_bf.append(np.ascontiguousarray(
            xT.reshape(KB, P, NCORES // 2, 512).transpose(2, 1, 0, 3)))
    Wq32 = np.asarray(Wq, np.float32)
    Wv32 = np.asarray(Wv, np.float32)
    host_bias = []

    def pmajor(w):
        """[D, N] -> [P, KB, N] with [p, ko, n] = w[ko*128+p, n]"""
        return np.ascontiguousarray(
            w.reshape(KB, P, w.shape[1]).transpose(1, 0, 2))

    for c in range(NCORES):
        b, g = divmod(c, G)
        cols = slice(g * GC, (g + 1) * GC)
        wq_c = Wqkv[:, 0 * D:1 * D][:, cols].astype(np.float32)
        wk_c = Wqkv[:, 1 * D:2 * D][:, cols]
        wv_c = Wqkv[:, 2 * D:3 * D][:, cols].astype(np.float32)
        # fold the per-group query/value projections (+ attention scale)
        # on host:
        wqs = Wq32[g] * SCALE
        wq_fold = (wq_c.reshape(D, HPG, HD) @ wqs[None]).reshape(D, GC)
        wv_fold = (wv_c.reshape(D, HPG, HD) @ Wv32[g][None]).reshape(D, GC)
        wqk = np.concatenate([wq_fold.astype(bf), np.asarray(wk_c, bf)],
                             axis=1)
        bq1 = bqkv[0 * D:1 * D][cols].astype(np.float32)
        bk1 = bqkv[1 * D:2 * D][cols].astype(np.float32)
        bv1 = bqkv[2 * D:3 * D][cols].astype(np.float32)
        bq2 = (bq1.reshape(HPG, HD) @ wqs
               + np.asarray(bq, np.float32)[g] * SCALE).reshape(GC)
        b1cat = np.concatenate([bq2, bk1]).astype(np.float32)
        host_bias.append(
            (bv1.reshape(HPG, HD) @ Wv32[g]
             + np.asarray(bv, np.float32)[g][None, :]).reshape(GC))
        in_maps.append({
            "xt": x_bf[b],
            "wqk": pmajor(wqk),
            "wvf": pmajor(wv_fold.astype(bf)),
            "b1": np.ascontiguousarray(b1cat.reshape(8, P).T),
            "onesc": np.ones((P, 1), bf),
        })

    res = run_bass_kernel_spmd(nc, in_maps, list(range(NCORES)),
                               **_CACHE.get("run_kwargs", {}))
    _CACHE["last_results"] = res

    out = np.empty((B, S, D), np.float32)
    for c in range(NCORES):
        b, g = divmod(c, G)
        o = np.asarray(res.results[c]["out"], np.float32)  # [GC,S] out^T
        den = res.results[c]["den"].reshape(HPG, S)        # [HPG,4,512]
        o = o / np.repeat(den, HD, axis=0)  # normalize rows h*128+e by den[h]
        o = o + host_bias[c][:, None]
        out[b, :, g * GC:(g + 1) * GC] = o.T
    return out


# revision 70
# speedup vs baseline: 1.0188x; 1.0047x over previous
"""GQA dense-transformer kernel for 8 Trainium2 NeuronCores.

Problem (hardcoded): B=2, S=2048, D=2048, kv_heads=16, groups G=4, HPG=4,
HD=128.  reference:
    qkv = x @ Wqkv + bqkv ; q,k,v = split(qkv)
    q = einsum('bsghd,gde->bsghe', q, Wq) + bq   (per-group shared proj)
    v = einsum('bsghd,gde->bsghe', v, Wv) + bv
    scores = einsum('bqghd,bkghd->bghqk', q, k) / sqrt(HD)
    attn = softmax(scores) * attn_mask           (mask == ones at grading)
    out = einsum('bghqk,bkghd->bqghd', attn, v)  -> [B,S,D]

Sharding: core c = b*4 + g handles (batch b, group g): it computes the
512 output columns [g*512,(g+1)*512) of out[b].

Per-core device program (bf16 matmuls, fp32 PSUM):
  BOTH per-group projections are folded into Wqkv on the HOST:
    W_q2'[:,hs] = Wqkv_q[:,hs] @ (Wq[g]*SCALE)   (query path + attn scale)
    W_v2'[:,hs] = Wqkv_v[:,hs] @ Wv[g]           (value path)
  so the device computes q2^T, k^T and v2 directly in one projection
  pass.  Inputs are RELAID OUT partition-major on the host so every DMA
  descriptor covers a 1-16KB contiguous run.
  phase 1: chunk 0 (s in [0,512)) runs K-OUTER in groups of 4 k-blocks
           with 8 live PSUM banks (m=0..7 = q2',k); v2 and chunks 1..3
           run m-outer.  v2 is computed in NATURAL (key-partition)
           layout directly: per 128-key block, lhsT = the xT key slice
           (stationary), rhs = W_v2' k-slab, PSUM-accumulated over the
           16 k-blocks.  q2/k evacuate on DVE (tensor_scalar bias add),
           v2 on DVE, keeping ACT free for early attention.
  early attention: scores+exp for heads 0/1, k-blocks j<12 (chunks
           0..2), query columns [0,1536) are INTERLEAVED into chunk 3's
           projection window, so the ACT engine (the phase-2
           bottleneck) starts ~60us early and phase 2 shrinks.
  phase 2: a single SLOT SCHEDULER walks (head, j) score slots -- for
           early (h,j) only the last query chunk remains -- and weaves
           the global PV queue (4 col-chunk matmuls per (h,j), j-MAJOR
           PSUM accumulation, 2-slot lag behind the exps) plus the
           denominator accumulation into the stream, so the PE and DVE
           track ACT with no per-head barrier.
           Denominators: dacc[h] += P_j on the DVE (bf16, 2x rate),
           column-summed by 4 CONCURRENT M=1 ones-matmuls in 4 PE
           col-groups (~0.25us PE per head).
           Output is UNNORMALIZED bf16 out^T + per-head denominator
           rows; softmax division + v-path bias happen on host.
"""
import sys
import numpy as np

sys.path.insert(0, "/opt/trn_rl_repo")
import ml_dtypes  # noqa: E402

B, S, D = 2, 2048, 2048
G, HPG, HD = 4, 4, 128
GC = HPG * HD            # 512 columns per group
SCALE = HD ** -0.5
P = 128
KB = D // P              # 16 contraction blocks
SB = S // P              # 16 sk blocks
NCORES = 8

_CACHE: dict = {}

# early-attention config: head -> number of leading k-blocks whose
# qc<3 scores+exps run inside chunk 3's projection window
NE = {0: 12, 1: 10}
EQ = 1536                # early query columns (qc 0..2)


def _build_program():
    import concourse.tile_sem_assignment as tsa
    # Walrus caps sync waits per instruction; _split_excess_waits breaks
    # any multi-wait compute instruction into standalone EventSemaphore
    # CTRLs on the same engine.  Keep the default 8 HWDGE semaphores so
    # DMA-completion waits stay fine-grained.
    tsa.NUM_HWDGE_SEMS = 8

    import concourse.bass as bass
    import concourse.tile as tile
    from concourse import mybir
    from contextlib import ExitStack

    bf16 = mybir.dt.bfloat16
    f32 = mybir.dt.float32

    nc = bass.Bass(trn_type="TRN2")
    SCH = 512                 # s-chunk width for projection phase
    NCH = S // SCH            # 4 chunks
    QCH = 1024                # sq chunk width for scores/exp
    xt_d = nc.dram_tensor("xt", [NCH, P, KB, SCH], bf16, kind="ExternalInput")
    wqk_d = nc.dram_tensor("wqk", [P, KB, 2 * GC], bf16, kind="ExternalInput")
    wvf_d = nc.dram_tensor("wvf", [P, KB, GC], bf16, kind="ExternalInput")
    b1_d = nc.dram_tensor("b1", [P, 8], f32, kind="ExternalInput")
    onesc_d = nc.dram_tensor("onesc", [P, 1], bf16, kind="ExternalInput")
    out_d = nc.dram_tensor("out", [GC, S], bf16, kind="ExternalOutput")
    den_d = nc.dram_tensor("den", [HPG, 4, SCH], f32, kind="ExternalOutput")

    Exp = mybir.ActivationFunctionType.Exp
    Add = mybir.AluOpType.add

    # early units (h, j, qc) in emission order: h0 fully, then h1
    early_units = [(h, j, qc) for h in (0, 1)
                   for qc in range(3) for j in range(NE[h])]

    with tile.TileContext(nc) as tc:
        with ExitStack() as octx:
            # ---- persistent tiles ----
            persist = octx.enter_context(tc.tile_pool(name="persist", bufs=1))
            k_sb = persist.tile([P, HPG, S], bf16)       # k^T per head
            q2_sb = persist.tile([P, HPG, S], bf16)      # q2^T per head
            v2_sb = persist.tile([P, SB, GC], bf16)      # v2 natural blocks
            b1_sb = persist.tile([P, 8], f32)
            ones_sb = persist.tile([P, 1], bf16)
            nc.sync.dma_start(b1_sb[:], b1_d[:])
            nc.sync.dma_start(ones_sb[:], onesc_d[:])
            # early exp tiles (qc 0..2 only; live chunk 3 -> their PV)
            pearly = octx.enter_context(tc.tile_pool(name="pearly", bufs=1))
            PET = {h: [pearly.tile([P, EQ], bf16, tag=f"pe{h}_{j}",
                                   name=f"pe{h}_{j}") for j in range(NE[h])]
                   for h in NE}
            # head-2 j=2,3 full score tiles, exp'd inside chunk 3's
            # v-window (their inputs are all evacuated by then)
            pfxA = octx.enter_context(tc.tile_pool(name="pfxA", bufs=1))
            VFWD = {j: pfxA.tile([P, S], bf16, tag=f"fx{j}", name=f"vf{j}")
                    for j in (2, 3, 4)}

            # ---------------- phase 1: projections ----------------
            with ExitStack() as ctx:
                wpool = ctx.enter_context(tc.tile_pool(name="w1", bufs=1))
                xpool = ctx.enter_context(tc.tile_pool(name="xT", bufs=2))
                tpool = ctx.enter_context(tc.tile_pool(name="tmp", bufs=1))
                wqk_sb = wpool.tile([P, KB, 2 * GC], bf16)
                wvf_sb = wpool.tile([P, KB, GC], bf16)

                def wslice(m, k):
                    return wqk_sb[:, k, m * P:(m + 1) * P]

                Ident = mybir.ActivationFunctionType.Identity

                def evac(ps, m, c, on_act=False):
                    """psum -> q2/k SBUF with bias add (DVE or ACT)."""
                    dst = q2_sb if m < 4 else k_sb
                    sl = dst[:, m % 4, c * SCH:(c + 1) * SCH]
                    if on_act:
                        nc.scalar.activation(sl, ps[:], Ident,
                                             bias=b1_sb[:, m:m + 1])
                    else:
                        nc.vector.tensor_scalar_add(sl, ps[:],
                                                    b1_sb[:, m:m + 1])

                # stage A: chunk 0, m=0..7 (q2', k), K-OUTER in groups of
                # 4 k-blocks with 8 live PSUM banks.
                # ONE phase-1 psum pool with per-bank tags: stage B tiles
                # reuse stage A's banks tag-by-tag, so their WAR waits
                # only on that single bank's evacuation instead of on a
                # fresh-pool barrier over all 8 (a ~4us PE bubble).
                ph1ps = ctx.enter_context(
                    tc.tile_pool(name="ph1ps", bufs=1, space="PSUM"))
                xT0 = xpool.tile([P, KB, SCH], bf16, tag="xT")
                if True:
                    psA = [ph1ps.tile([P, SCH], f32, tag=f"b{m}",
                                      name=f"psA{m}")
                           for m in range(8)]
                    # warm the PE during the initial DMA window: dummy
                    # matmuls keep the HAM clock gate at 8/8 so the first
                    # real matmuls run at 2.4GHz.  17 matmuls ~ 5us,
                    # matching the arrival of the first k-block's data.
                    ws = tpool.tile([P, SCH], bf16, tag="ws", name="ws")
                    nc.any.memset(ws[:], 0.0)
                    for _ in range(17):
                        nc.tensor.matmul(psA[0][:], ws[:, 0:P], ws[:],
                                         start=True, stop=True)
                    # first k-block split across 4 queues so the first
                    # real matmul's ~0.4MB lands as early as possible
                    nc.sync.dma_start(wqk_sb[:, 0, 0:512], wqk_d[:, 0, 0:512])
                    nc.sync.dma_start(wqk_sb[:, 0, 512:1024],
                                      wqk_d[:, 0, 512:1024])
                    nc.sync.dma_start(xT0[:, 0], xt_d[0, :, 0])
                    kgroups = [(1, 2), (2, 3), (3, 4), (4, 6), (6, 8),
                               (8, 12), (12, 16)]
                    for lo, hi in kgroups:
                        ks = slice(lo, hi)
                        nc.sync.dma_start(wqk_sb[:, ks], wqk_d[:, ks])
                        nc.sync.dma_start(xT0[:, ks], xt_d[0, :, ks])
                    # 4 pieces on separate queues: one 2MB DMA on a
                    # single engine (~40us) would stall the chunk-0
                    # v-blocks by ~4us.
                    for g4 in range(4):
                        ks = slice(4 * g4, 4 * g4 + 4)
                        nc.sync.dma_start(wvf_sb[:, ks], wvf_d[:, ks])
                    for k in range(KB):
                        for m in range(8):
                            nc.tensor.matmul(
                                psA[m][:], wslice(m, k), xT0[:, k],
                                start=(k == 0), stop=(k == KB - 1))
                    for m in range(8):
                        # alternate DVE/ACT so the 8 serialized copies
                        # drain in ~3us instead of ~6us
                        evac(psA[m], m, 0, on_act=(m % 2 == 1))

                # stage B: v2 chunk 0, then all m for chunks 1..3, with
                # the early attention units woven into chunk 3's window.
                if True:
                    def vblock(xT, c, sb):
                        vt = ph1ps.tile([P, GC], f32, tag=f"b{sb % 2}",
                                        name="vt")
                        for k in range(KB):
                            nc.tensor.matmul(
                                vt[:], xT[:, k, sb * P:(sb + 1) * P],
                                wvf_sb[:, k, :],
                                start=(k == 0), stop=(k == KB - 1))
                        nc.vector.tensor_copy(
                            v2_sb[:, c * (SCH // P) + sb, :], vt[:])

                    eu_cnt = [0]

                    ESB = [5, 6]

                    def escore(h, j, qc, dst):
                        eu_cnt[0] += 1
                        ss = ph1ps.tile([P, SCH], f32,
                                        tag=f"b{ESB[eu_cnt[0] % len(ESB)]}",
                                        name="ess")
                        nc.tensor.matmul(
                            ss[:], k_sb[:, h, j * P:(j + 1) * P],
                            q2_sb[:, h, qc * SCH:(qc + 1) * SCH],
                            start=True, stop=True)
                        nc.scalar.activation(
                            dst[:, qc * SCH:(qc + 1) * SCH], ss[:], Exp)

                    def early_unit(h, j, qc):
                        escore(h, j, qc, PET[h][j])

                    for sb in range(SCH // P):
                        vblock(xT0, 0, sb)
                    for c in range(1, NCH):
                        xT = xpool.tile([P, KB, SCH], bf16, tag="xT")
                        # 4 k-range pieces on separate queues
                        for g4 in range(4):
                            ks = slice(4 * g4, 4 * g4 + 4)
                            nc.sync.dma_start(xT[:, ks], xt_d[c, :, ks])
                        ew = list(early_units) if c == NCH - 1 else []
                        for m in range(8):
                            ps = ph1ps.tile([P, SCH], f32,
                                            tag=f"b{2 + m % 3}", name="pp")
                            for k in range(KB):
                                nc.tensor.matmul(
                                    ps[:], wslice(m, k), xT[:, k],
                                    start=(k == 0), stop=(k == KB - 1))
                            evac(ps, m, c)
                            for _ in range(6):
                                if ew:
                                    early_unit(*ew.pop(0))
                        vf = [(j, qc) for j in (2, 3, 4)
                              for qc in range(4)] if c == NCH - 1 else []
                        if c == NCH - 1:
                            ESB[:] = [2, 3, 5, 6]
                        for sb in range(SCH // P):
                            last = sb == SCH // P - 1
                            if last:
                                # emit the final quota BEFORE the last
                                # v-block so its matmuls hide the exp
                                # drain and phase 2 starts barrier-free
                                while ew:
                                    early_unit(*ew.pop(0))
                                while vf:
                                    j, qc = vf.pop(0)
                                    escore(2, j, qc, VFWD[j])
                            vblock(xT, c, sb)
                            if not last:
                                for _ in range(5):
                                    if ew:
                                        early_unit(*ew.pop(0))
                                for _ in range(3):
                                    if vf:
                                        j, qc = vf.pop(0)
                                        escore(2, j, qc, VFWD[j])

            # ---------------- phase 2: attention ----------------
            with ExitStack() as ctx:
                ppool = ctx.enter_context(tc.tile_pool(name="P", bufs=10))
                pfx = ctx.enter_context(tc.tile_pool(name="pfx", bufs=1))
                q3pool = ctx.enter_context(tc.tile_pool(name="q3", bufs=11))
                opool = ctx.enter_context(tc.tile_pool(name="osb", bufs=4))
                dpool = ctx.enter_context(tc.tile_pool(name="dsb", bufs=1))
                dapool = ctx.enter_context(tc.tile_pool(name="dacc", bufs=1))
                sps = ctx.enter_context(
                    tc.tile_pool(name="sps", bufs=2, space="PSUM"))
                ops = ctx.enter_context(
                    tc.tile_pool(name="ops", bufs=1, space="PSUM"))

                # slot list: for early (h,j) only the qc3 scores remain.
                # Within a head, interleave the ACT-heavy full slots among
                # the PE-heavy q3 slots so neither engine stalls.
                slots = []
                for h in range(HPG):
                    ne = NE.get(h, 0)
                    q3l = [(h, j, True) for j in range(ne)]
                    ful = [(h, j, False) for j in range(ne, SB)]
                    nf, nq = len(ful), len(q3l)
                    fi = qi = 0
                    for t in range(nf + nq):
                        if qi >= nq or (fi < nf
                                        and fi * (nf + nq) <= t * nf):
                            slots.append(ful[fi])
                            fi += 1
                        else:
                            slots.append(q3l[qi])
                            qi += 1
                # the h0/h1 region is PE-bound (PV-heavy, exps mostly done
                # early) while h2/h3's is ACT-bound: forward 3 of h2's
                # score slots (exp-heavy; their PV stays in h2's window)
                # into the early region to balance both.
                FWD = [(2, j, False) for j in range(2)]
                for s in FWD:
                    slots.remove(s)
                for pos, s in zip((18, 8), FWD[::-1]):
                    slots.insert(pos, s)
                # j=2..4 were scored+exp'd in chunk 3's v-window
                for j in (2, 3, 4):
                    slots.remove((2, j, False))

                Q3T = {}          # (h,j) -> [P,512] qc3 exp tile
                PF = {}           # (h,j) -> [P,S] full exp tile
                dacc = {}         # h -> [P,S] bf16 denominator accum
                po = {}           # h -> 4 live PV psum banks
                ready = {}        # (h,j) -> slot idx of its last exp
                for j in (2, 3, 4):
                    PF[(2, j)] = VFWD[j]
                    ready[(2, j)] = -10 ** 9

                def pv_group(h, j):
                    early = j < NE.get(h, 0)
                    for qc in range(4):
                        if early and qc < 3:
                            rhs = PET[h][j][:, qc * 512:(qc + 1) * 512]
                        elif early:
                            rhs = Q3T[(h, j)][:]
                        else:
                            rhs = PF[(h, j)][:, qc * 512:(qc + 1) * 512]
                        nc.tensor.matmul(
                            po[h][qc][:], v2_sb[:, j, h * HD:(h + 1) * HD],
                            rhs, start=(j == 0), stop=(j == SB - 1))

                Ident2 = mybir.ActivationFunctionType.Identity

                def pv_den(h):
                    """PV epilogue + denominators for head h."""
                    for qc in range(4):
                        sl = slice(qc * 512, (qc + 1) * 512)
                        osb = opool.tile([P, 512], bf16, tag="o", name="osb")
                        # alternate DVE/ACT so the 4 serialized copies
                        # drain twice as fast (matters for the last head)
                        if qc % 2 == 0:
                            nc.vector.tensor_copy(osb[:], po[h][qc][:])
                        else:
                            nc.scalar.activation(osb[:], po[h][qc][:],
                                                 Ident2)
                        nc.sync.dma_start(out_d[h * P:(h + 1) * P, sl], osb[:])
                    # dacc (bf16) column-summed by 4 CONCURRENT M=1
                    # ones-matmuls in 4 PE col-groups
                    pd = ops.tile([P, 512], f32, tag="po0", name="pd")
                    for qc in range(4):
                        nc.tensor.matmul(
                            pd[32 * qc:32 * qc + 1, :],
                            ones_sb[:, 0:1],
                            dacc[h][:, qc * 512:(qc + 1) * 512],
                            start=True, stop=True,
                            tile_position=(0, 32 * qc))
                    d4 = dpool.tile([P, 512], f32, tag="dp", name="d4")
                    nc.vector.tensor_copy(d4[:], pd[:])
                    # single strided DMA: partitions {0,32,64,96} -> den[h]
                    d4v = d4[:].rearrange("(a b) f -> a b f", b=32)[:, 0:1, :]
                    nc.sync.dma_start(den_d[h], d4v)

                pvq = [(h, j) for h in range(HPG) for j in range(SB)]
                pvi = [0]
                LAG2 = 2

                def drain_pv(si, budget):
                    while pvi[0] < len(pvq) and budget > 0:
                        hp, jp = pvq[pvi[0]]
                        if ready.get((hp, jp), 10 ** 9) > si - LAG2:
                            return
                        if jp == 0:
                            po[hp] = [ops.tile([P, 512], f32, tag=f"po{qc}",
                                               name=f"po{qc}")
                                      for qc in range(4)]
                        pv_group(hp, jp)
                        if jp == SB - 1:
                            pv_den(hp)
                        pvi[0] += 1
                        budget -= 1

                for si, (h, j, is_q3) in enumerate(slots):
                    drain_pv(si, 1)
                    if (h, j) == (2, 5):
                        # fold the v-window-scored j=2..4 into h2's dacc
                        for jj in (2, 3, 4):
                            nc.vector.tensor_tensor(
                                dacc[2][:], dacc[2][:], VFWD[jj][:], op=Add)
                    first = h not in dacc
                    if first:
                        dacc[h] = dapool.tile([P, S], bf16, tag=f"da{h}",
                                              name=f"da{h}")
                    if is_q3:
                        ss = sps.tile([P, QCH], f32, name="ss")
                        nc.tensor.matmul(
                            ss[:, 0:512], k_sb[:, h, j * P:(j + 1) * P],
                            q2_sb[:, h, EQ:S], start=True, stop=True)
                        q3t = q3pool.tile([P, 512], bf16, tag="q3",
                                          name="q3t")
                        nc.scalar.activation(q3t[:], ss[:, 0:512], Exp)
                        Q3T[(h, j)] = q3t
                        # denominator accumulation (DVE, bf16 2x rate)
                        if first:
                            nc.vector.tensor_copy(
                                dacc[h][:, 0:EQ], PET[h][j][:])
                            nc.vector.tensor_copy(
                                dacc[h][:, EQ:S], q3t[:])
                        else:
                            nc.vector.tensor_tensor(
                                dacc[h][:, 0:EQ], dacc[h][:, 0:EQ],
                                PET[h][j][:], op=Add)
                            nc.vector.tensor_tensor(
                                dacc[h][:, EQ:S], dacc[h][:, EQ:S],
                                q3t[:], op=Add)
                    else:
                        if (h, j, False) in FWD:
                            # forwarded slots live much longer than the
                            # ppool rotation: dedicated buffers
                            Pj = pfx.tile([P, S], bf16, tag=f"fx{j}",
                                          name="Pfx")
                        else:
                            Pj = ppool.tile([P, S], bf16, tag="P", name="Pj")
                        for qc in range(S // QCH):
                            ss = sps.tile([P, QCH], f32, name="ss")
                            for half in range(QCH // 512):
                                off = qc * QCH + half * 512
                                nc.tensor.matmul(
                                    ss[:, half * 512:(half + 1) * 512],
                                    k_sb[:, h, j * P:(j + 1) * P],
                                    q2_sb[:, h, off:off + 512],
                                    start=True, stop=True)
                            nc.scalar.activation(
                                Pj[:, qc * QCH:(qc + 1) * QCH], ss[:], Exp)
                        PF[(h, j)] = Pj
                        if first:
                            nc.vector.tensor_copy(dacc[h][:], Pj[:])
                        elif si == len(slots) - 1:
                            # last slot: slice the add per query-chunk so
                            # the concurrent pd col-group matmuls can
                            # start before the full add finishes
                            for qc in range(4):
                                sl = slice(qc * 512, (qc + 1) * 512)
                                nc.vector.tensor_tensor(
                                    dacc[h][:, sl], dacc[h][:, sl],
                                    Pj[:, sl], op=Add)
                        else:
                            nc.vector.tensor_tensor(
                                dacc[h][:], dacc[h][:], Pj[:], op=Add)
                    ready[(h, j)] = si
                    drain_pv(si, 1)
                drain_pv(10 ** 9, 10 ** 9)

    _split_excess_waits(nc, mybir)
    return nc


def _split_excess_waits(nc, mybir):
    """Each TPB instruction has ONE wait slot (NEURON_ISA_TPB_EVENTS); walrus
    refuses instructions with more sync waits.  Tile attaches the full
    vector-clock wait list to instructions, so split all but one wait out
    into standalone EventSemaphore (CTRL) instructions on the same engine,
    placed immediately before.  Semantics are identical: all waits must be
    satisfied before the instruction executes."""
    import copy
    template = None
    for blk in nc.m.functions[0].blocks:
        for inst in blk.instructions:
            if isinstance(inst, mybir.InstEventSemaphore):
                template = inst
                break
        if template is not None:
            break
    assert template is not None, "no EventSemaphore template found"
    uid = [0]
    for fn in nc.m.functions:
        for blk in fn.blocks:
            out = []
            for inst in blk.instructions:
                si = inst.sync_info
                if si is not None and len(si.on_wait) > 1:
                    waits = list(si.on_wait)
                    for w in waits[:-1]:
                        ev = copy.deepcopy(template)
                        ev.name = f"swsplit-{uid[0]}"
                        uid[0] += 1
                        ev.engine = inst.engine
                        ev.sync_info = mybir.SyncInfo(on_wait=[w], on_update=[])
                        out.append(ev)
                    si.on_wait = waits[-1:]
                    inst.sync_info = si
                out.append(inst)
            blk.instructions[:] = out
    return nc


def _numpy_fallback(x, attn_mask, Wqkv, bqkv, Wq, bq, Wv, bv):
    x = np.asarray(x, np.float32)
    qkv = x @ np.asarray(Wqkv, np.float32) + np.asarray(bqkv, np.float32)
    q, k, v = np.split(qkv, 3, axis=-1)
    q = q.reshape(B, S, G, HPG, HD)
    k = k.reshape(B, S, G, HPG, HD)
    v = v.reshape(B, S, G, HPG, HD)
    q = np.einsum('bsghd,gde->bsghe', q, np.asarray(Wq, np.float32)) \
        + np.asarray(bq, np.float32)[None, None, :, None, :]
    v = np.einsum('bsghd,gde->bsghe', v, np.asarray(Wv, np.float32)) \
        + np.asarray(bv, np.float32)[None, None, :, None, :]
    out = np.empty((B, S, G, HPG, HD), np.float32)
    for b in range(B):
        for g in range(G):
            for hh in range(HPG):
                s = (q[b, :, g, hh] @ k[b, :, g, hh].T) * SCALE
                s = s - s.max(axis=-1, keepdims=True)
                p = np.exp(s)
                p /= p.sum(axis=-1, keepdims=True)
                p = p * np.asarray(attn_mask, np.float32)
                out[b, :, g, hh] = p @ v[b, :, g, hh]
    return out.reshape(B, S, D)


def kernel(x, attn_mask, Wqkv, bqkv, Wq, bq, Wv, bv):
    x = np.asarray(x)
    attn_mask = np.asarray(attn_mask)
    Wqkv = np.asarray(Wqkv)
    bqkv = np.asarray(bqkv)
    Wq = np.asarray(Wq)
    bq = np.asarray(bq)
    Wv = np.asarray(Wv)
    bv = np.asarray(bv)

    if not np.all(attn_mask == 1.0):
        # general (non-ones) post-softmax mask: correct but slow host path
        return _numpy_fallback(x, attn_mask, Wqkv, bqkv, Wq, bq, Wv, bv)

    if "nc" not in _CACHE:
        _CACHE["nc"] = _build_program()
    nc = _CACHE["nc"]
    from concourse.bass_utils import run_bass_kernel_spmd

    bf = ml_dtypes.bfloat16
    in_maps = []
    # xt layout [chunk, p, ko, s']: xt[c,p,ko,s'] = x[b][c*512+s', ko*128+p]
    x_bf = []
    for b in range(B):
        xT = np.asarray(x[b], np.float32).T.astype(bf)      # [D, S]
        x_bf.append(np.ascontiguousarray(
            xT.reshape(KB, P, NCORES // 2, 512).transpose(2, 1, 0, 3)))
    Wq32 = np.asarray(Wq, np.float32)
    Wv32 = np.asarray(Wv, np.float32)
    host_bias = []

    def pmajor(w):
        """[D, N] -> [P, KB, N] with [p, ko, n] = w[ko*128+p, n]"""
        return np.ascontiguousarray(
            w.reshape(KB, P, w.shape[1]).transpose(1, 0, 2))

    for c in range(NCORES):
        b, g = divmod(c, G)
        cols = slice(g * GC, (g + 1) * GC)
        wq_c = Wqkv[:, 0 * D:1 * D][:, cols].astype(np.float32)
        wk_c = Wqkv[:, 1 * D:2 * D][:, cols]
        wv_c = Wqkv[:, 2 * D:3 * D][:, cols].astype(np.float32)
        # fold the per-group query/value projections (+ attention scale)
        # on host:
        wqs = Wq32[g] * SCALE
        wq_fold = (wq_c.reshape(D, HPG, HD) @ wqs[None]).reshape(D, GC)
        wv_fold = (wv_c.reshape(D, HPG, HD) @ Wv32[g][None]).reshape(D, GC)
        wqk = np.concatenate([wq_fold.astype(bf), np.asarray(wk_c, bf)],
                             axis=1)
        bq1 = bqkv[0 * D:1 * D][cols].astype(np.float32)
        bk1 = bqkv[1 * D:2 * D][cols].astype(np.float32)
        bv1 = bqkv[2 * D:3 * D][cols].astype(np.float32)
        bq2 = (bq1.reshape(HPG, HD) @ wqs
               + np.asarray(bq, np.float32)[g] * SCALE).reshape(GC)
        b1cat = np.concatenate([bq2, bk1]).astype(np.float32)
        host_bias.append(
            (bv1.reshape(HPG, HD) @ Wv32[g]
             + np.asarray(bv, np.float32)[g][None, :]).reshape(GC))
        in_maps.append({
            "xt": x_bf[b],
            "wqk": pmajor(wqk),
            "wvf": pmajor(wv_fold.astype(bf)),
            "b1": np.ascontiguousarray(b1cat.reshape(8, P).T),
            "onesc": np.ones((P, 1), bf),
        })

    res = run_bass_kernel_spmd(nc, in_maps, list(range(NCORES)),
                               **_CACHE.get("run_kwargs", {}))
    _CACHE["last_results"] = res

    out = np.empty((B, S, D), np.float32)
    for c in range(NCORES):
        b, g = divmod(c, G)
        o = np.asarray(res.results[c]["out"], np.float32)  # [GC,S] out^T
        den = res.results[c]["den"].reshape(HPG, S)        # [HPG,4,512]
        o = o / np.repeat(den, HD, axis=0)  # normalize rows h*128+e by den[h]
        o = o + host_bias[c][:, None]
        out[b, :, g * GC:(g + 1) * GC] = o.T
    return out
